# revision 15
# baseline (speedup 1.0000x reference)
"""Distributed causal multi-head attention on one TRN2 chip (8 NeuronCores).

Problem: B=2, S=2048, D=1024, H=16, DH=64 (f32), causal softmax attention with
QKV + output projections.

Sharding (SPMD, one Bass graph for all 8 cores):
  core i -> batch b = i // 4, head group g = i % 4 (4 of 16 heads).
Each core projects Q/K/V for its 4 heads over the full sequence of its batch
and runs causal attention.  Per-head z (bf16) is AllGathered within each
batch's 4-core group one 512-row band at a time; each core then computes a
256-column slice of the output projection.  Core (b, g) returns
out[b, :, 256g:256g+256]; the host concatenates.

v2 design notes (perf):
  - host prepacks x/w so every SBUF load is ONE wide contiguous DMA
    (the v1 kernel issued 185 DMAs serially at ~600ns each on the Sync queue)
  - a tiny warmup AllGather at kernel start absorbs the CC-stream bootstrap
    (~35us of first-collective overhead in v1)
  - exp is batched 2 key-chunks per ACTIVATE ([128,1024] from a 2-bank PSUM
    tile) to amortize the ~350-cycle ACT startup
  - causal mask is multiplicative post-exp (enables exp batching and diagonal
    trimming); diagonal chunks only compute the causally-needed query width
  - softmax normalization: reciprocal_approx_fast (v1 used the 8-cycle/elem
    iterative DVE reciprocal on a single-lane [1,512] tile = 3.3us each) +
    gpsimd partition_broadcast (v1 burned PE matmuls on the broadcast)
  - emission is software-pipelined: projection of band t+1 and output
    projection of band t-1 are emitted as fillers inside attention of band t
    so the PE never idles (HAM clock gate re-throttles to 1.2GHz after any
    ~3.4us PE-idle window)
  - PSUM budget (8 banks): psc 2x[128,1024]f32 (4) + pz 2x[128,512] (2) +
    aux 2x[128,512] (2); aux quanta are self-contained (matmuls + copy-out)
"""

import sys

for _p in ("/opt/trn_rl_repo", "/opt/pypackages"):
    if _p not in sys.path:
        sys.path.insert(0, _p)

from collections import deque
from contextlib import ExitStack

import numpy as np

import concourse.bass as bass
import concourse.mybir as mybir
import concourse.tile as tile
from concourse import bacc
from concourse.bass_utils import run_bass_kernel_spmd

B, S, D, H, DH = 2, 2048, 1024, 16, 64
G = 4                       # heads per core
NCORES = 8
SCALE = float(np.sqrt(DH))
TQ = 512                    # query tile (free dim)
NQT = S // TQ               # 4
KC = 128                    # key chunk (partition dim)
DC = 128                    # contraction d-chunk
NDC = D // DC               # 8
EG = G * DH                 # 256: packed head dim per group
VW = DH + 1                 # 65: head slot width in v_aug (ones column)
DS = D // 4                 # 256: output D-column slice per core
XW = NQT * NDC * TQ         # 16384: prepacked x row length
TRIM = True                 # trim diagonal score/AV matmuls to causal width

F32 = mybir.dt.float32
BF16 = mybir.dt.bfloat16

EXP = mybir.ActivationFunctionType.Exp

GROUPS = [[0, 1, 2, 3], [4, 5, 6, 7]]

_CACHE = {}


def _build() -> bass.Bass:
    nc = bacc.Bacc("TRN2", num_devices=NCORES, target_bir_lowering=False)

    xq = nc.declare_dram_parameter("xq", [DC, XW], BF16, isOutput=False)
    xk = nc.declare_dram_parameter("xk", [DC, XW], BF16, isOutput=False)
    xv = nc.declare_dram_parameter("xv", [DC, XW], BF16, isOutput=False)
    wq = nc.declare_dram_parameter("wq", [DC, NDC * EG], BF16, isOutput=False)
    wk = nc.declare_dram_parameter("wk", [DC, NDC * EG], BF16, isOutput=False)
    wv = nc.declare_dram_parameter("wv", [DC, NDC * EG], BF16, isOutput=False)
    wo = nc.declare_dram_parameter("wo", [DC, NDC * DS], BF16, isOutput=False)
    mask = nc.declare_dram_parameter("mask", [KC, G * TQ], BF16, isOutput=False)
    out_ext = nc.declare_dram_parameter("out", [S, DS], F32, isOutput=True)

    with ExitStack() as ctx:
        tc = ctx.enter_context(tile.TileContext(nc))
        const = ctx.enter_context(tc.tile_pool(name="const", bufs=1))
        dram = ctx.enter_context(tc.tile_pool(name="dram", bufs=1, space="DRAM"))
        xpool = ctx.enter_context(tc.tile_pool(name="x", bufs=2))
        epool = ctx.enter_context(tc.tile_pool(name="e", bufs=3))
        rpool = ctx.enter_context(tc.tile_pool(name="r", bufs=2))
        zgpool = ctx.enter_context(tc.tile_pool(name="zg", bufs=2))
        opool = ctx.enter_context(tc.tile_pool(name="o", bufs=2))
        psc_p = ctx.enter_context(tc.tile_pool(name="psc", bufs=2, space="PSUM"))
        pz_p = ctx.enter_context(tc.tile_pool(name="pz", bufs=2, space="PSUM"))
        aux_p = ctx.enter_context(tc.tile_pool(name="aux", bufs=2, space="PSUM"))

        # ---- CC warmup: a tiny AllGather so the collectives stream is
        # bootstrapped while the projections run (first real gather then runs
        # at steady-state speed) ----
        win = nc.dram_tensor("cc_warm_in", [1, 2], F32)
        wout = nc.dram_tensor("cc_warm_out", [G, 2], F32)
        nc.gpsimd.collective_compute(
            "AllGather",
            mybir.AluOpType.bypass,
            replica_groups=GROUPS,
            ins=[win.ap()],
            outs=[wout.ap()],
        )

        # ---- constants (one wide DMA each; host prepacked) ----
        wq_sb = const.tile([DC, NDC * EG], BF16, name="wq_sb")
        wk_sb = const.tile([DC, NDC * EG], BF16, name="wk_sb")
        wv_sb = const.tile([DC, NDC * EG], BF16, name="wv_sb")
        wo_sb = const.tile([DC, NDC * DS], BF16, name="wo_sb")
        mask_sb = const.tile([KC, G * TQ], BF16, name="mask_sb")

        def load_w(dst, src_, pieces=2):
            wd = dst.shape[1] // pieces
            for i in range(pieces):
                nc.sync.dma_start(dst[:, i * wd:(i + 1) * wd],
                                  src_[:, i * wd:(i + 1) * wd])

        # v_aug: per k-chunk, per head: 64 value cols + 1 ones col
        vaug = const.tile([KC, (S // KC) * G * VW], BF16, name="vaug")
        nc.gpsimd.memset(vaug[:], 1.0)

        q_sb = [const.tile([2 * DH, S], BF16, name=f"q_sb{p}") for p in range(2)]
        k_sb = [const.tile([2 * DH, S], BF16, name=f"k_sb{p}") for p in range(2)]
        z_sb = [const.tile([2 * DH, S], BF16, name=f"z_sb{p}") for p in range(2)]

        # zero the psc banks once: with diagonal trimming, unwritten columns
        # are read by exp (exp(0)=1, then multiplied by the 0 mask)
        for _ in range(2):
            t_ = psc_p.tile([KC, 2 * TQ], F32, tag="psc", name="psc_init")
            nc.vector.memset(t_[:], 0.0)

        # ---- x band staging (double-buffered, one DMA per input per band) ----
        xb = {}

        def load_x_one(nm, src_, t, pieces=4):
            b_ = xpool.tile([DC, NDC * TQ], BF16, tag=f"x{nm}", name=f"x{nm}{t}")
            w4 = NDC * TQ // pieces
            for s4 in range(pieces):
                nc.sync.dma_start(
                    b_[:, s4 * w4:(s4 + 1) * w4],
                    src_[:, t * NDC * TQ + s4 * w4:
                         t * NDC * TQ + (s4 + 1) * w4],
                )
            xb[(nm, t)] = b_

        def load_x_band(t):
            load_x_one("q", xq, t)
            load_x_one("k", xk, t)
            load_x_one("v", xv, t)

        # startup order matches the k-first quantum order
        load_w(wk_sb, wk)
        load_x_one("k", xk, 0)
        load_w(wv_sb, wv)
        load_x_one("v", xv, 0)
        load_w(wq_sb, wq)
        load_x_one("q", xq, 0)
        load_w(wo_sb, wo)
        nc.sync.dma_start(mask_sb[:], mask[:, :])
        load_x_band(1)

        # ---- projection quanta (self-contained: psum alloc + mm + copy) ----
        def q_or_k_quantum(t, p, xkey, wsb, dst):
            def run():
                acc = aux_p.tile([KC, TQ], F32, tag="aux", name="acc")
                xt = xb[(xkey, t)]
                for c in range(NDC):
                    nc.tensor.matmul(
                        acc[:],
                        wsb[:, c * EG + p * 128: c * EG + (p + 1) * 128],
                        xt[:, c * TQ:(c + 1) * TQ],
                        start=(c == 0),
                        stop=(c == NDC - 1),
                    )
                nc.vector.tensor_copy(dst[p][:, t * TQ:(t + 1) * TQ], acc[:])
            return run

        def v_quantum(t, sub):
            def run():
                acc = aux_p.tile([KC, TQ], F32, tag="aux", name="accv")
                xt = xb[("v", t)]
                for c in range(NDC):
                    nc.tensor.matmul(
                        acc[:, 0:EG],
                        xt[:, c * TQ + sub * KC: c * TQ + (sub + 1) * KC],
                        wv_sb[:, c * EG:(c + 1) * EG],
                        start=(c == 0),
                        stop=(c == NDC - 1),
                    )
                kci = t * 4 + sub
                base = kci * G * VW
                dst = vaug[:, base:base + G * VW].rearrange(
                    "p (h w) -> p h w", h=G
                )[:, :, 0:DH]
                src = acc[:, 0:EG].rearrange("p (h w) -> p h w", h=G)
                nc.vector.tensor_copy(dst, src)
            return run

        def proj_quanta(t):
            # K first, then V, then Q: at position 2 the attention of band 3
            # consumes band 2's K (scores, from step 5) and V (AV, step 6+)
            # while these quanta are being popped one per step -- K/V must be
            # emitted before their first consumer or the PE queue deadlocks.
            qs = []
            for p in range(2):
                qs.append(q_or_k_quantum(t, p, "k", wk_sb, k_sb))
            for sub in range(4):
                qs.append(v_quantum(t, sub))
            for p in range(2):
                qs.append(q_or_k_quantum(t, p, "q", wq_sb, q_sb))
            return qs

        # ---- per-band DRAM staging for the z AllGather (one gather per
        # band: the CC stream is serial with ~7us fixed cost per op, so
        # fewer/bigger gathers beat split halves) ----
        zb = [dram.tile([2 * KC, TQ], BF16, name=f"zb{t}") for t in range(NQT)]
        zg = [dram.tile([G * EG, TQ], BF16, name=f"zg{t}") for t in range(NQT)]

        def stage_and_gather(t):
            for p in range(2):
                nc.gpsimd.dma_start(
                    zb[t][p * KC:(p + 1) * KC, :],
                    z_sb[p][:, t * TQ:(t + 1) * TQ],
                )
            nc.gpsimd.collective_compute(
                "AllGather",
                mybir.AluOpType.bypass,
                replica_groups=GROUPS,
                ins=[zb[t].opt()],
                outs=[zg[t].opt()],
            )

        # ---- output projection quanta ----
        def oproj_quanta(t):
            state = {}

            def first():
                zt = zgpool.tile([KC, NDC * TQ], BF16, tag="zg", name="zg_sb")
                nc.sync.dma_start(
                    zt[:].rearrange("p (c j) -> p c j", c=NDC),
                    zg[t][:, :].rearrange("(c p) j -> p c j", c=NDC),
                )
                state["zg"] = zt
                state["o"] = opool.tile([KC, 4 * DS], F32, tag="o", name="o_sb")

            def qs_quantum(qs):
                def run():
                    if qs == 0:
                        first()
                    zt, o_sb = state["zg"], state["o"]
                    acc = aux_p.tile([KC, TQ], F32, tag="aux", name="acco")
                    for c in range(NDC):
                        nc.tensor.matmul(
                            acc[:, 0:DS],
                            zt[:, c * TQ + qs * KC: c * TQ + (qs + 1) * KC],
                            wo_sb[:, c * DS:(c + 1) * DS],
                            start=(c == 0),
                            stop=(c == NDC - 1),
                        )
                    nc.vector.tensor_copy(
                        o_sb[:, qs * DS:(qs + 1) * DS], acc[:, 0:DS]
                    )
                    if qs == 3:
                        nc.sync.dma_start(
                            out_ext[t * TQ:(t + 1) * TQ, :].rearrange(
                                "(q p) j -> p q j", q=4
                            ),
                            o_sb[:].rearrange("p (q j) -> p q j", q=4),
                        )
                return run

            return [qs_quantum(qs) for qs in range(4)]

        # ---- attention band with interleaved fillers ----
        def normalize(t, h, pz):
            p_i, off = h // 2, (h % 2) * DH
            # den lives at psum partition 64; the reciprocal_approx_fast
            # custom-DVE op needs a partition-0 SBUF source (it read garbage
            # from partition 64) and gpsimd cannot read PSUM, so: copy den to
            # sbuf, fast-reciprocal there, gpsimd-broadcast, multiply.
            den_s = rpool.tile([1, TQ], F32, tag="den", name="den_s")
            nc.vector.tensor_copy(den_s[:], pz[DH:DH + 1, :])
            recip = rpool.tile([1, TQ], F32, tag="recip", name="recip")
            nc.vector.reciprocal_approx_fast(recip[:], den_s[:])
            bc = rpool.tile([DH, TQ], F32, tag="bc", name="bc")
            nc.gpsimd.partition_broadcast(bc[:], recip[:])
            nc.vector.tensor_mul(
                z_sb[p_i][off:off + DH, t * TQ:(t + 1) * TQ],
                pz[0:DH, :], bc[:]
            )

        def attention_band(t, dq_proj, dq_oproj):
            nkc = 4 * t + 4
            ngrp = nkc // 2
            steps_total = G * ngrp
            step = 0
            pending = None     # (h, g, closure, pz) AV one group behind

            def col0(kci):
                dc = kci - 4 * t
                return max(dc, 0) * KC if TRIM else 0

            for h in range(G):
                p_i, off = h // 2, (h % 2) * DH
                pz = pz_p.tile([KC, TQ], F32, tag="pz", name=f"pz{h}")
                for g in range(ngrp):
                    # scores for chunks 2g, 2g+1 into a 2-bank psc tile
                    psc = psc_p.tile([KC, 2 * TQ], F32, tag="psc", name="psc")
                    for i in range(2):
                        kci = 2 * g + i
                        c0 = col0(kci)
                        nc.tensor.matmul(
                            psc[:, i * TQ + c0:(i + 1) * TQ],
                            k_sb[p_i][off:off + DH, kci * KC:(kci + 1) * KC],
                            q_sb[p_i][off:off + DH, t * TQ + c0:(t + 1) * TQ],
                            start=True,
                            stop=True,
                        )
                    e_t = epool.tile([KC, 2 * TQ], BF16, tag="e", name="e_t")
                    nc.scalar.activation(e_t[:], psc[:], EXP)
                    if 2 * g >= 4 * t:      # diagonal pair: multiplicative mask
                        mg = g - 2 * t
                        em = epool.tile(
                            [KC, 2 * TQ], BF16, tag="em", bufs=2, name="em"
                        )
                        nc.vector.tensor_mul(
                            em[:], e_t[:],
                            mask_sb[:, mg * 2 * TQ:(mg + 1) * 2 * TQ],
                        )
                        e_use = em
                    else:
                        e_use = e_t

                    def av(h=h, g=g, e_use=e_use, pz=pz):
                        for i in range(2):
                            kci = 2 * g + i
                            c0 = col0(kci)
                            nc.tensor.matmul(
                                pz[0:VW, c0:TQ],
                                vaug[:, kci * G * VW + h * VW:
                                     kci * G * VW + (h + 1) * VW],
                                e_use[:, i * TQ + c0:(i + 1) * TQ],
                                start=(kci == 0),
                                stop=(kci == nkc - 1),
                            )

                    # one filler quantum between scores(g) and AV(g-1)
                    step += 1
                    if dq_proj:
                        dq_proj.popleft()()
                    elif dq_oproj and step > (3 * steps_total) // 4:
                        dq_oproj.popleft()()
                    if pending is not None:
                        ph, _, pav, ppz = pending
                        pav()
                        if pending[1] == ngrp - 1:    # head ph's last group
                            normalize(t, ph, ppz)
                    pending = (h, g, av, pz)

            # flush the last AV + normalize, then any leftover fillers
            ph, _, pav, ppz = pending
            pav()
            normalize(t, ph, ppz)
            stage_and_gather(t)
            for dq in (dq_proj, dq_oproj):
                while dq:
                    dq.popleft()()

        # ---- band order: cheap band 2 last, so the final gather triggers
        # as early as possible and band 2's attention hosts late oproj work.
        # pos0 proj'd in prologue; during pos i we project band at pos i+1;
        # oproj(band) runs two positions after its gathers fire. ----
        BAND_ORDER = (0, 1, 3, 2)
        for qm in proj_quanta(BAND_ORDER[0]):
            qm()

        for i, t in enumerate(BAND_ORDER):
            if 1 <= i < NQT - 1:
                load_x_band(BAND_ORDER[i + 1])
            dq_proj = (deque(proj_quanta(BAND_ORDER[i + 1]))
                       if i + 1 < NQT else deque())
            dq_oproj = deque()
            if i == 2:
                dq_oproj.extend(oproj_quanta(BAND_ORDER[0]))
            elif i == 3:
                dq_oproj.extend(oproj_quanta(BAND_ORDER[1]))
                dq_oproj.extend(oproj_quanta(BAND_ORDER[2]))
            attention_band(t, dq_proj, dq_oproj)

        # ---- tail: output projection of the last-processed band ----
        for qm in oproj_quanta(BAND_ORDER[3]):
            qm()

    nc.compile()
    return nc


def _get_graph() -> bass.Bass:
    if "nc" not in _CACHE:
        _CACHE["nc"] = _build()
    return _CACHE["nc"]


def _make_mask() -> np.ndarray:
    # multiplicative causal mask for the 4 diagonal chunks of a band:
    # m[x, dc*TQ + y] = 1.0 where key dc*KC+x <= query y else 0.0
    m = np.zeros((KC, G * TQ), np.float32)
    x = np.arange(KC)[:, None]
    y = np.arange(TQ)[None, :]
    for dc in range(G):
        m[:, dc * TQ:(dc + 1) * TQ] = (dc * KC + x <= y).astype(np.float32)
    return m


def _make_in_maps(inputs: dict) -> list[dict]:
    import ml_dtypes

    bf16 = ml_dtypes.bfloat16
    qx = np.asarray(inputs["query_input"], np.float32)
    kx = np.asarray(inputs["key_input"], np.float32)
    vx = np.asarray(inputs["value_input"], np.float32)
    WQ = (np.asarray(inputs["W_Q"], np.float32) / SCALE).astype(bf16)
    WK = np.asarray(inputs["W_K"], np.float32).astype(bf16)
    WV = np.asarray(inputs["W_V"], np.float32).astype(bf16)
    WO = np.asarray(inputs["W_O"], np.float32).astype(bf16)

    mask = _make_mask().astype(bf16)

    # x prepack: x_prep[p, (t*NDC + c)*TQ + j] = x[b, t*TQ + j, c*DC + p]
    def prep_x(arr, b):
        a = arr[b].astype(bf16)                       # [S, D]
        a = a.reshape(NQT, TQ, NDC, DC)               # [t, j, c, p]
        a = a.transpose(3, 0, 2, 1)                   # [p, t, c, j]
        return np.ascontiguousarray(a.reshape(DC, XW))

    xT = {
        (nm, b): prep_x(arr, b)
        for nm, arr in (("xq", qx), ("xk", kx), ("xv", vx))
        for b in range(B)
    }

    # weights: w_prep[p, c*EG + m] = W2[c*DC + p, m], W2 = [D, EG] head-packed
    def prep_w(w, hs):
        W2 = w[hs].transpose(1, 0, 2).reshape(D, EG)  # [d, h*DH + e]
        W2 = W2.reshape(NDC, DC, EG).transpose(1, 0, 2)
        return np.ascontiguousarray(W2.reshape(DC, NDC * EG))

    WO_flat = WO.reshape(H * DH, D)   # e' = h*64 + e, h-major (AllGather order)
    wmaps = []
    for g in range(G):
        hs = slice(g * G, (g + 1) * G)
        wo_slice = WO_flat[:, g * DS:(g + 1) * DS]    # [D, DS]
        wo_prep = np.ascontiguousarray(
            wo_slice.reshape(NDC, DC, DS).transpose(1, 0, 2).reshape(
                DC, NDC * DS)
        )
        wmaps.append(
            {
                "wq": prep_w(WQ, hs),
                "wk": prep_w(WK, hs),
                "wv": prep_w(WV, hs),
                "wo": wo_prep,
            }
        )

    in_maps = []
    for core in range(NCORES):
        b, g = core // G, core % G
        m = {
            "xq": xT[("xq", b)],
            "xk": xT[("xk", b)],
            "xv": xT[("xv", b)],
            "mask": mask,
        }
        m.update(wmaps[g])
        in_maps.append(m)
    return in_maps


def _assemble(results: list[dict]) -> np.ndarray:
    out = np.empty((B, S, D), np.float32)
    for core in range(NCORES):
        b, g = core // G, core % G
        out[b, :, g * DS:(g + 1) * DS] = results[core]["out"]
    return out


def run(inputs: dict, trace: bool = False):
    """Run on hardware; returns (output, BassKernelResults)."""
    nc = _get_graph()
    res = run_bass_kernel_spmd(
        nc, _make_in_maps(inputs), core_ids=list(range(NCORES)), trace=trace
    )
    return _assemble(res.results), res


def kernel(**inputs) -> np.ndarray:
    out, _ = run(inputs)
    return out


# revision 18
# speedup vs baseline: 1.0177x; 1.0177x over previous
"""Distributed causal multi-head attention on one TRN2 chip (8 NeuronCores).

Problem: B=2, S=2048, D=1024, H=16, DH=64 (f32), causal softmax attention with
QKV + output projections.

Sharding (SPMD, one Bass graph for all 8 cores):
  core i -> batch b = i // 4, head group g = i % 4 (4 of 16 heads).
Each core projects Q/K/V for its 4 heads over the full sequence of its batch
and runs causal attention.  Per-head z (bf16) is AllGathered within each
batch's 4-core group one 512-row band at a time; each core then computes a
256-column slice of the output projection.  Core (b, g) returns
out[b, :, 256g:256g+256]; the host concatenates.

v2 design notes (perf):
  - host prepacks x/w so every SBUF load is ONE wide contiguous DMA
    (the v1 kernel issued 185 DMAs serially at ~600ns each on the Sync queue)
  - a tiny warmup AllGather at kernel start absorbs the CC-stream bootstrap
    (~35us of first-collective overhead in v1)
  - exp is batched 2 key-chunks per ACTIVATE ([128,1024] from a 2-bank PSUM
    tile) to amortize the ~350-cycle ACT startup
  - causal mask is multiplicative post-exp (enables exp batching and diagonal
    trimming); diagonal chunks only compute the causally-needed query width
  - softmax normalization: reciprocal_approx_fast (v1 used the 8-cycle/elem
    iterative DVE reciprocal on a single-lane [1,512] tile = 3.3us each) +
    gpsimd partition_broadcast (v1 burned PE matmuls on the broadcast)
  - emission is software-pipelined: projection of band t+1 and output
    projection of band t-1 are emitted as fillers inside attention of band t
    so the PE never idles (HAM clock gate re-throttles to 1.2GHz after any
    ~3.4us PE-idle window)
  - PSUM budget (8 banks): psc 2x[128,1024]f32 (4) + pz 2x[128,512] (2) +
    aux 2x[128,512] (2); aux quanta are self-contained (matmuls + copy-out)
"""

import sys

for _p in ("/opt/trn_rl_repo", "/opt/pypackages"):
    if _p not in sys.path:
        sys.path.insert(0, _p)

from collections import deque
from contextlib import ExitStack

import numpy as np

import concourse.bass as bass
import concourse.mybir as mybir
import concourse.tile as tile
from concourse import bacc
from concourse.bass_utils import run_bass_kernel_spmd

B, S, D, H, DH = 2, 2048, 1024, 16, 64
G = 4                       # heads per core
NCORES = 8
SCALE = float(np.sqrt(DH))
TQ = 512                    # query tile (free dim)
NQT = S // TQ               # 4
KC = 128                    # key chunk (partition dim)
DC = 128                    # contraction d-chunk
NDC = D // DC               # 8
EG = G * DH                 # 256: packed head dim per group
VW = DH + 1                 # 65: head slot width in v_aug (ones column)
DS = D // 4                 # 256: output D-column slice per core
XW = NQT * NDC * TQ         # 16384: prepacked x row length
TRIM = True                 # trim diagonal score/AV matmuls to causal width

F32 = mybir.dt.float32
F32R = mybir.dt.float32r
BF16 = mybir.dt.bfloat16

EXP = mybir.ActivationFunctionType.Exp

GROUPS = [[0, 1, 2, 3], [4, 5, 6, 7]]

_CACHE = {}


def _build() -> bass.Bass:
    nc = bacc.Bacc("TRN2", num_devices=NCORES, target_bir_lowering=False)

    xq = nc.declare_dram_parameter("xq", [DC, XW], BF16, isOutput=False)
    xk = nc.declare_dram_parameter("xk", [DC, XW], BF16, isOutput=False)
    xv = nc.declare_dram_parameter("xv", [DC, XW], BF16, isOutput=False)
    wq = nc.declare_dram_parameter("wq", [DC, NDC * EG], BF16, isOutput=False)
    wk = nc.declare_dram_parameter("wk", [DC, NDC * EG], BF16, isOutput=False)
    wv = nc.declare_dram_parameter("wv", [DC, NDC * EG], BF16, isOutput=False)
    wo = nc.declare_dram_parameter("wo", [DC, NDC * DS], BF16, isOutput=False)
    mask = nc.declare_dram_parameter("mask", [KC, G * TQ], BF16, isOutput=False)
    out_ext = nc.declare_dram_parameter("out", [S, DS], F32, isOutput=True)

    with ExitStack() as ctx:
        tc = ctx.enter_context(tile.TileContext(nc))
        const = ctx.enter_context(tc.tile_pool(name="const", bufs=1))
        dram = ctx.enter_context(tc.tile_pool(name="dram", bufs=1, space="DRAM"))
        xpool = ctx.enter_context(tc.tile_pool(name="x", bufs=2))
        epool = ctx.enter_context(tc.tile_pool(name="e", bufs=3))
        rpool = ctx.enter_context(tc.tile_pool(name="r", bufs=2))
        zgpool = ctx.enter_context(tc.tile_pool(name="zg", bufs=2))
        opool = ctx.enter_context(tc.tile_pool(name="o", bufs=2))
        psc_p = ctx.enter_context(tc.tile_pool(name="psc", bufs=2, space="PSUM"))
        pz_p = ctx.enter_context(tc.tile_pool(name="pz", bufs=2, space="PSUM"))
        aux_p = ctx.enter_context(tc.tile_pool(name="aux", bufs=2, space="PSUM"))

        # ---- CC warmup: a tiny AllGather so the collectives stream is
        # bootstrapped while the projections run (first real gather then runs
        # at steady-state speed) ----
        win = nc.dram_tensor("cc_warm_in", [1, 2], F32)
        wout = nc.dram_tensor("cc_warm_out", [G, 2], F32)
        nc.gpsimd.collective_compute(
            "AllGather",
            mybir.AluOpType.bypass,
            replica_groups=GROUPS,
            ins=[win.ap()],
            outs=[wout.ap()],
        )

        # ---- constants (one wide DMA each; host prepacked) ----
        wq_sb = const.tile([DC, NDC * EG], BF16, name="wq_sb")
        wk_sb = const.tile([DC, NDC * EG], BF16, name="wk_sb")
        wv_sb = const.tile([DC, NDC * EG], BF16, name="wv_sb")
        wo_sb = const.tile([DC, NDC * DS], BF16, name="wo_sb")
        mask_sb = const.tile([KC, G * TQ], BF16, name="mask_sb")

        def load_w(dst, src_, pieces=2):
            wd = dst.shape[1] // pieces
            for i in range(pieces):
                nc.sync.dma_start(dst[:, i * wd:(i + 1) * wd],
                                  src_[:, i * wd:(i + 1) * wd])

        # v_aug: per k-chunk, per head: 64 value cols + 1 ones col
        vaug = const.tile([KC, (S // KC) * G * VW], BF16, name="vaug")
        nc.gpsimd.memset(vaug[:], 1.0)
        ones_b = const.tile([1, DH], BF16, name="ones_b")
        nc.vector.memset(ones_b[:], 1.0)

        q_sb = [const.tile([2 * DH, S], BF16, name=f"q_sb{p}") for p in range(2)]
        k_sb = [const.tile([2 * DH, S], BF16, name=f"k_sb{p}") for p in range(2)]
        z_sb = [const.tile([2 * DH, S], BF16, name=f"z_sb{p}") for p in range(2)]

        # zero the psc banks once: with diagonal trimming, unwritten columns
        # are read by exp (exp(0)=1, then multiplied by the 0 mask)
        for _ in range(2):
            t_ = psc_p.tile([KC, 2 * TQ], F32, tag="psc", name="psc_init")
            nc.vector.memset(t_[:], 0.0)

        # ---- x band staging (double-buffered, one DMA per input per band) ----
        xb = {}

        def load_x_one(nm, src_, t, pieces=4):
            b_ = xpool.tile([DC, NDC * TQ], BF16, tag=f"x{nm}", name=f"x{nm}{t}")
            w4 = NDC * TQ // pieces
            for s4 in range(pieces):
                nc.sync.dma_start(
                    b_[:, s4 * w4:(s4 + 1) * w4],
                    src_[:, t * NDC * TQ + s4 * w4:
                         t * NDC * TQ + (s4 + 1) * w4],
                )
            xb[(nm, t)] = b_

        def load_x_band(t):
            load_x_one("q", xq, t)
            load_x_one("k", xk, t)
            load_x_one("v", xv, t)

        # startup order matches the k-first quantum order
        load_w(wk_sb, wk)
        load_x_one("k", xk, 0)
        load_w(wv_sb, wv)
        load_x_one("v", xv, 0)
        load_w(wq_sb, wq)
        load_x_one("q", xq, 0)
        load_w(wo_sb, wo)
        nc.sync.dma_start(mask_sb[:], mask[:, :])
        load_x_band(1)

        # ---- projection quanta (self-contained: psum alloc + mm + copy) ----
        def q_or_k_quantum(t, p, xkey, wsb, dst):
            def run():
                acc = aux_p.tile([KC, TQ], F32, tag="aux", name="acc")
                xt = xb[(xkey, t)]
                for c in range(NDC):
                    nc.tensor.matmul(
                        acc[:],
                        wsb[:, c * EG + p * 128: c * EG + (p + 1) * 128],
                        xt[:, c * TQ:(c + 1) * TQ],
                        start=(c == 0),
                        stop=(c == NDC - 1),
                    )
                nc.vector.tensor_copy(dst[p][:, t * TQ:(t + 1) * TQ], acc[:])
            return run

        def v_quantum(t, sub):
            def run():
                acc = aux_p.tile([KC, TQ], F32, tag="aux", name="accv")
                xt = xb[("v", t)]
                for c in range(NDC):
                    nc.tensor.matmul(
                        acc[:, 0:EG],
                        xt[:, c * TQ + sub * KC: c * TQ + (sub + 1) * KC],
                        wv_sb[:, c * EG:(c + 1) * EG],
                        start=(c == 0),
                        stop=(c == NDC - 1),
                    )
                kci = t * 4 + sub
                base = kci * G * VW
                dst = vaug[:, base:base + G * VW].rearrange(
                    "p (h w) -> p h w", h=G
                )[:, :, 0:DH]
                src = acc[:, 0:EG].rearrange("p (h w) -> p h w", h=G)
                nc.vector.tensor_copy(dst, src)
            return run

        def proj_quanta(t):
            # K first, then V, then Q: at position 2 the attention of band 3
            # consumes band 2's K (scores, from step 5) and V (AV, step 6+)
            # while these quanta are being popped one per step -- K/V must be
            # emitted before their first consumer or the PE queue deadlocks.
            qs = []
            for p in range(2):
                qs.append(q_or_k_quantum(t, p, "k", wk_sb, k_sb))
            for sub in range(4):
                qs.append(v_quantum(t, sub))
            for p in range(2):
                qs.append(q_or_k_quantum(t, p, "q", wq_sb, q_sb))
            return qs

        # ---- per-band DRAM staging for the z AllGather (one gather per
        # band: the CC stream is serial with ~7us fixed cost per op, so
        # fewer/bigger gathers beat split halves) ----
        zb = [dram.tile([2 * KC, TQ], BF16, name=f"zb{t}") for t in range(NQT)]
        zg = [dram.tile([G * EG, TQ], BF16, name=f"zg{t}") for t in range(NQT)]

        def stage_and_gather(t):
            for p in range(2):
                nc.sync.dma_start(
                    zb[t][p * KC:(p + 1) * KC, :],
                    z_sb[p][:, t * TQ:(t + 1) * TQ],
                )
            nc.gpsimd.collective_compute(
                "AllGather",
                mybir.AluOpType.bypass,
                replica_groups=GROUPS,
                ins=[zb[t].opt()],
                outs=[zg[t].opt()],
            )

        # ---- output projection quanta ----
        def oproj_quanta(t):
            state = {}

            def first():
                zt = zgpool.tile([KC, NDC * TQ], BF16, tag="zg", name="zg_sb")
                nc.sync.dma_start(
                    zt[:].rearrange("p (c j) -> p c j", c=NDC),
                    zg[t][:, :].rearrange("(c p) j -> p c j", c=NDC),
                )
                state["zg"] = zt
                state["o"] = opool.tile([KC, 4 * DS], F32, tag="o", name="o_sb")

            def qs_quantum(qs):
                def run():
                    if qs == 0:
                        first()
                    zt, o_sb = state["zg"], state["o"]
                    acc = aux_p.tile([KC, TQ], F32, tag="aux", name="acco")
                    for c in range(NDC):
                        nc.tensor.matmul(
                            acc[:, 0:DS],
                            zt[:, c * TQ + qs * KC: c * TQ + (qs + 1) * KC],
                            wo_sb[:, c * DS:(c + 1) * DS],
                            start=(c == 0),
                            stop=(c == NDC - 1),
                        )
                    nc.vector.tensor_copy(
                        o_sb[:, qs * DS:(qs + 1) * DS], acc[:, 0:DS]
                    )
                    if qs == 3:
                        nc.sync.dma_start(
                            out_ext[t * TQ:(t + 1) * TQ, :].rearrange(
                                "(q p) j -> p q j", q=4
                            ),
                            o_sb[:].rearrange("p (q j) -> p q j", q=4),
                        )
                return run

            return [qs_quantum(qs) for qs in range(4)]

        # ---- attention band with interleaved fillers ----
        def normalize(t, h, pz):
            p_i, off = h // 2, (h % 2) * DH
            # den lives at psum partition 64; the reciprocal_approx_fast
            # custom-DVE op needs a partition-0 SBUF source (it read garbage
            # from partition 64) and gpsimd cannot read PSUM, so: copy den to
            # sbuf, fast-reciprocal there, gpsimd-broadcast, multiply.
            den_s = rpool.tile([1, TQ], F32, tag="den", name="den_s")
            nc.vector.tensor_copy(den_s[:], pz[DH:DH + 1, :])
            recip = rpool.tile([1, TQ], F32, tag="recip", name="recip")
            nc.vector.reciprocal_approx_fast(recip[:], den_s[:])
            recip_b = rpool.tile([1, TQ], BF16, tag="recipb", name="recip_b")
            with nc.allow_low_precision(reason="softmax denom recip, bf16"):
                nc.vector.tensor_copy(recip_b[:], recip[:])
            # broadcast recip across 64 partitions on the PE (bf16 rank-1
            # matmul); keeps gpsimd free for the collectives whose CC-cores
            # run there
            pb = aux_p.tile([KC, TQ], F32, tag="aux", name="pb")
            nc.tensor.matmul(
                pb[0:DH, :], ones_b[:], recip_b[:],
                start=True, stop=True,
            )
            bc_s = rpool.tile([DH, TQ], F32, tag="bc", name="bc_s")
            nc.vector.tensor_copy(bc_s[:], pb[0:DH, :])
            nc.vector.tensor_mul(
                z_sb[p_i][off:off + DH, t * TQ:(t + 1) * TQ],
                pz[0:DH, :], bc_s[:]
            )

        def attention_band(t, dq_proj, dq_oproj):
            nkc = 4 * t + 4
            ngrp = nkc // 2
            steps_total = G * ngrp
            step = 0
            pending = None     # (h, g, closure, pz) AV one group behind

            def col0(kci):
                dc = kci - 4 * t
                return max(dc, 0) * KC if TRIM else 0

            for h in range(G):
                p_i, off = h // 2, (h % 2) * DH
                pz = pz_p.tile([KC, TQ], F32, tag="pz", name=f"pz{h}")
                for g in range(ngrp):
                    # scores for chunks 2g, 2g+1 into a 2-bank psc tile
                    psc = psc_p.tile([KC, 2 * TQ], F32, tag="psc", name="psc")
                    for i in range(2):
                        kci = 2 * g + i
                        c0 = col0(kci)
                        nc.tensor.matmul(
                            psc[:, i * TQ + c0:(i + 1) * TQ],
                            k_sb[p_i][off:off + DH, kci * KC:(kci + 1) * KC],
                            q_sb[p_i][off:off + DH, t * TQ + c0:(t + 1) * TQ],
                            start=True,
                            stop=True,
                        )
                    e_t = epool.tile([KC, 2 * TQ], BF16, tag="e", name="e_t")
                    nc.scalar.activation(e_t[:], psc[:], EXP)
                    if 2 * g >= 4 * t:      # diagonal pair: multiplicative mask
                        mg = g - 2 * t
                        em = epool.tile(
                            [KC, 2 * TQ], BF16, tag="em", bufs=2, name="em"
                        )
                        nc.vector.tensor_mul(
                            em[:], e_t[:],
                            mask_sb[:, mg * 2 * TQ:(mg + 1) * 2 * TQ],
                        )
                        e_use = em
                    else:
                        e_use = e_t

                    def av(h=h, g=g, e_use=e_use, pz=pz):
                        for i in range(2):
                            kci = 2 * g + i
                            c0 = col0(kci)
                            nc.tensor.matmul(
                                pz[0:VW, c0:TQ],
                                vaug[:, kci * G * VW + h * VW:
                                     kci * G * VW + (h + 1) * VW],
                                e_use[:, i * TQ + c0:(i + 1) * TQ],
                                start=(kci == 0),
                                stop=(kci == nkc - 1),
                            )

                    # one filler quantum between scores(g) and AV(g-1)
                    step += 1
                    if dq_proj:
                        dq_proj.popleft()()
                    elif dq_oproj and step > (3 * steps_total) // 4:
                        dq_oproj.popleft()()
                    if pending is not None:
                        ph, _, pav, ppz = pending
                        pav()
                        if pending[1] == ngrp - 1:    # head ph's last group
                            normalize(t, ph, ppz)
                    pending = (h, g, av, pz)

            # flush the last AV + normalize, then any leftover fillers
            ph, _, pav, ppz = pending
            pav()
            normalize(t, ph, ppz)
            stage_and_gather(t)
            for dq in (dq_proj, dq_oproj):
                while dq:
                    dq.popleft()()

        # ---- band order: cheap band 2 last, so the final gather triggers
        # as early as possible and band 2's attention hosts late oproj work.
        # pos0 proj'd in prologue; during pos i we project band at pos i+1;
        # oproj(band) runs two positions after its gathers fire. ----
        BAND_ORDER = (0, 1, 3, 2)
        for qm in proj_quanta(BAND_ORDER[0]):
            qm()

        for i, t in enumerate(BAND_ORDER):
            if 1 <= i < NQT - 1:
                load_x_band(BAND_ORDER[i + 1])
            dq_proj = (deque(proj_quanta(BAND_ORDER[i + 1]))
                       if i + 1 < NQT else deque())
            dq_oproj = deque()
            if i == 2:
                dq_oproj.extend(oproj_quanta(BAND_ORDER[0]))
            elif i == 3:
                dq_oproj.extend(oproj_quanta(BAND_ORDER[1]))
            attention_band(t, dq_proj, dq_oproj)

        # ---- tail: band 3's oproj runs while band 2's gather is in flight,
        # then band 2's oproj ----
        for qm in oproj_quanta(BAND_ORDER[2]):
            qm()
        for qm in oproj_quanta(BAND_ORDER[3]):
            qm()

    nc.compile()
    return nc


def _get_graph() -> bass.Bass:
    if "nc" not in _CACHE:
        _CACHE["nc"] = _build()
    return _CACHE["nc"]


def _make_mask() -> np.ndarray:
    # multiplicative causal mask for the 4 diagonal chunks of a band:
    # m[x, dc*TQ + y] = 1.0 where key dc*KC+x <= query y else 0.0
    m = np.zeros((KC, G * TQ), np.float32)
    x = np.arange(KC)[:, None]
    y = np.arange(TQ)[None, :]
    for dc in range(G):
        m[:, dc * TQ:(dc + 1) * TQ] = (dc * KC + x <= y).astype(np.float32)
    return m


def _make_in_maps(inputs: dict) -> list[dict]:
    import ml_dtypes

    bf16 = ml_dtypes.bfloat16
    qx = np.asarray(inputs["query_input"], np.float32)
    kx = np.asarray(inputs["key_input"], np.float32)
    vx = np.asarray(inputs["value_input"], np.float32)
    WQ = (np.asarray(inputs["W_Q"], np.float32) / SCALE).astype(bf16)
    WK = np.asarray(inputs["W_K"], np.float32).astype(bf16)
    WV = np.asarray(inputs["W_V"], np.float32).astype(bf16)
    WO = np.asarray(inputs["W_O"], np.float32).astype(bf16)

    mask = _make_mask().astype(bf16)

    # x prepack: x_prep[p, (t*NDC + c)*TQ + j] = x[b, t*TQ + j, c*DC + p]
    def prep_x(arr, b):
        a = arr[b].astype(bf16)                       # [S, D]
        a = a.reshape(NQT, TQ, NDC, DC)               # [t, j, c, p]
        a = a.transpose(3, 0, 2, 1)                   # [p, t, c, j]
        return np.ascontiguousarray(a.reshape(DC, XW))

    xT = {
        (nm, b): prep_x(arr, b)
        for nm, arr in (("xq", qx), ("xk", kx), ("xv", vx))
        for b in range(B)
    }

    # weights: w_prep[p, c*EG + m] = W2[c*DC + p, m], W2 = [D, EG] head-packed
    def prep_w(w, hs):
        W2 = w[hs].transpose(1, 0, 2).reshape(D, EG)  # [d, h*DH + e]
        W2 = W2.reshape(NDC, DC, EG).transpose(1, 0, 2)
        return np.ascontiguousarray(W2.reshape(DC, NDC * EG))

    WO_flat = WO.reshape(H * DH, D)   # e' = h*64 + e, h-major (AllGather order)
    wmaps = []
    for g in range(G):
        hs = slice(g * G, (g + 1) * G)
        wo_slice = WO_flat[:, g * DS:(g + 1) * DS]    # [D, DS]
        wo_prep = np.ascontiguousarray(
            wo_slice.reshape(NDC, DC, DS).transpose(1, 0, 2).reshape(
                DC, NDC * DS)
        )
        wmaps.append(
            {
                "wq": prep_w(WQ, hs),
                "wk": prep_w(WK, hs),
                "wv": prep_w(WV, hs),
                "wo": wo_prep,
            }
        )

    in_maps = []
    for core in range(NCORES):
        b, g = core // G, core % G
        m = {
            "xq": xT[("xq", b)],
            "xk": xT[("xk", b)],
            "xv": xT[("xv", b)],
            "mask": mask,
        }
        m.update(wmaps[g])
        in_maps.append(m)
    return in_maps


def _assemble(results: list[dict]) -> np.ndarray:
    out = np.empty((B, S, D), np.float32)
    for core in range(NCORES):
        b, g = core // G, core % G
        out[b, :, g * DS:(g + 1) * DS] = results[core]["out"]
    return out


def run(inputs: dict, trace: bool = False):
    """Run on hardware; returns (output, BassKernelResults)."""
    nc = _get_graph()
    res = run_bass_kernel_spmd(
        nc, _make_in_maps(inputs), core_ids=list(range(NCORES)), trace=trace
    )
    return _assemble(res.results), res


def kernel(**inputs) -> np.ndarray:
    out, _ = run(inputs)
    return out
